# revision 1
# baseline (speedup 1.0000x reference)
"""GuidedAttentionLoss on 8 Trainium2 NeuronCores (Bass/Tile).

loss = mean(guide * a^T) over [B=64, T=2048, N=512], where
  guide[b,t,k] = (1 - exp(-((k - floor(N_b/T_b * t))/N_b)^2 / (2*sigma^2)))
                 for t < T_b, k < N_b; 0 elsewhere.

Strategy (pure data parallel, 8 batches per core):
  * Host knows the integer lengths, so the per-(b,t) guide coefficients are
    precomputed on host as tiny tensors:
      scaled squared distance  c*(n - o_t)^2 = n^2*R0[t] + n*R1[t] + R2[t]
    with R0=c, R1=-2c*o_t, R2=c*o_t^2 and L=[n^2, n, 1]; the PE computes each
    [128, w] "sq" tile as a K=3 fp32 matmul (L^T @ R) into PSUM.
  * ScalarE: e = Exp(-sq)  (single activation pass, PSUM->SBUF).
  * VectorE: one scalar_tensor_tensor per tile: out=(e-1)*a with accum_out
    giving per-partition sums of (e-1)*a == -a*guide.  That single op fuses
    the (1-e) fold, the product with a, and the reduction.
  * Host: loss = -sum(accums over cores) / (B*N*T) in f64.
  * Only the valid [N_b, T_b] rectangle is streamed; outside it the host
    zero-pads the staged input so padded/junk lanes contribute exactly 0.
  * One SPMD program for all 8 cores: the 64 batches are dealt into 8
    "slots" x 8 cores (swap hill-climb minimizing sum(maxN*maxT)) so every
    instruction's shape is the per-slot max — small padding, perfectly
    balanced cores.
"""

import numpy as np

B, N_MAX, T_MAX = 64, 512, 2048
SIGMA = 0.4
N_CORES = 8
PART = 128
CHUNK = 512  # max fp32 matmul moving free dim == one PSUM bank


def _plan(input_lengths: np.ndarray, target_lengths: np.ndarray):
    """Assign the B batches to (slot, core) so all cores share shapes.

    Returns list of (idxs[n_cores], tiles, T_slot) per slot, where tiles is
    the list of row-tile heights ([128, ..., partial]) covering max(N) of the
    slot and T_slot = max(T).  Assignment minimizes sum(maxN*maxT) (the
    per-core padded DMA volume) with a swap hill-climb from a sorted start.
    """
    Ns = input_lengths.astype(np.int64)
    Ts = target_lengths.astype(np.int64)
    assert Ns.shape == (B,) and Ts.shape == (B,)
    n_slots = B // N_CORES

    def slot_cost(g):
        return int(max(Ns[i] for i in g)) * int(max(Ts[i] for i in g))

    def sweep(groups):
        # full pairwise-swap local search to convergence
        improved = True
        while improved:
            improved = False
            for s1 in range(n_slots):
                for s2 in range(s1 + 1, n_slots):
                    g1, g2 = groups[s1], groups[s2]
                    for i1 in range(N_CORES):
                        for i2 in range(N_CORES):
                            c0 = slot_cost(g1) + slot_cost(g2)
                            g1[i1], g2[i2] = g2[i2], g1[i1]
                            if slot_cost(g1) + slot_cost(g2) < c0:
                                improved = True
                            else:
                                g1[i1], g2[i2] = g2[i2], g1[i1]
        return groups

    rng = np.random.default_rng(0)
    best_cost, groups = None, None
    for trial in range(12):
        if trial == 0:
            order = np.argsort(-(Ns * Ts))
        elif trial == 1:
            order = np.argsort(-Ts)
        elif trial == 2:
            order = np.argsort(-Ns)
        else:
            order = rng.permutation(B)
        cand = sweep(
            [list(order[s * N_CORES : (s + 1) * N_CORES]) for s in range(n_slots)]
        )
        c = sum(slot_cost(g) for g in cand)
        if best_cost is None or c < best_cost:
            best_cost, groups = c, [list(g) for g in cand]

    slots = []
    for g in groups:
        idxs = np.array(g)
        maxN = int(Ns[idxs].max())
        T_slot = int(Ts[idxs].max())
        tiles = [PART] * (maxN // PART)
        if maxN % PART:
            tiles.append(maxN % PART)
        slots.append((idxs, tiles, T_slot))
    return slots


def _host_inputs(alignments, input_lengths, target_lengths, slots):
    """Per-core input dicts for run_bass_kernel_spmd."""
    tot_rows = sum(sum(tiles) for _, tiles, _ in slots)
    n_slots = len(slots)
    t = np.arange(T_MAX, dtype=np.float32)

    lmat = np.zeros((3, N_MAX), dtype=np.float32)
    j = np.arange(N_MAX, dtype=np.float32)
    lmat[0] = j * j
    lmat[1] = j
    lmat[2] = 1.0

    in_maps = []
    for core in range(N_CORES):
        A = np.zeros((tot_rows, T_MAX), dtype=np.float32)
        R = np.zeros((n_slots * 3, T_MAX), dtype=np.float32)
        row0 = 0
        for s, (idxs, tiles, T_slot) in enumerate(slots):
            b = int(idxs[core])
            Nb = int(input_lengths[b])
            Tb = int(target_lengths[b])
            A[row0 : row0 + Nb, :Tb] = alignments[b, :Nb, :Tb]
            # match reference f32 arithmetic: floor(float32(N)/float32(T) * t)
            ratio = np.float32(Nb) / np.float32(Tb)
            o = np.floor(ratio * t)
            c = np.float32(1.0) / np.float32(2.0 * SIGMA * SIGMA * Nb * Nb)
            R[3 * s + 0] = c
            R[3 * s + 1] = np.float32(-2.0) * c * o
            R[3 * s + 2] = c * o * o
            row0 += sum(tiles)
        in_maps.append({"a": A, "r": R, "lmat": lmat})
    return in_maps


def _build_bass(slots, reps: int = 1):
    import concourse.bacc as bacc
    import concourse.mybir as mybir
    from concourse.tile import TileContext

    fp32 = mybir.dt.float32
    n_slots = len(slots)
    tot_rows = sum(sum(tiles) for _, tiles, _ in slots)
    n_units = sum(
        len(tiles) * (-(-T_slot // CHUNK)) for _, tiles, T_slot in slots
    )
    assert n_units <= PART, f"too many units for accumulator: {n_units}"

    nc = bacc.Bacc(
        "TRN2", target_bir_lowering=False, debug=False, num_devices=N_CORES
    )
    a_d = nc.dram_tensor("a", [tot_rows, T_MAX], fp32, kind="ExternalInput")
    r_d = nc.dram_tensor("r", [n_slots * 3, T_MAX], fp32, kind="ExternalInput")
    l_d = nc.dram_tensor("lmat", [3, N_MAX], fp32, kind="ExternalInput")
    oacc_d = nc.dram_tensor("out_acc", [PART, PART], fp32, kind="ExternalOutput")

    with TileContext(nc) as tc:
        with (
            tc.tile_pool(name="const", bufs=1) as constp,
            tc.tile_pool(name="apool", bufs=5) as apool,
            tc.tile_pool(name="rpool", bufs=3) as rpool,
            tc.tile_pool(name="epool", bufs=6) as epool,
            tc.tile_pool(name="mpool", bufs=3) as mpool,
            tc.tile_pool(name="accp", bufs=1) as accp,
            tc.tile_pool(name="sqp", bufs=7, space="PSUM") as sqp,
        ):
            l_sb = constp.tile([3, N_MAX], fp32, tag="lmat")
            nc.sync.dma_start(out=l_sb[:], in_=l_d.ap()[:])
            acc = accp.tile([PART, PART], fp32, tag="acc")
            nc.vector.memset(acc[:], 0.0)

            for _rep in range(reps):
                unit = 0
                row0 = 0
                for s, (_, tiles, T_slot) in enumerate(slots):
                    r_sb = rpool.tile([3, T_MAX], fp32, tag="r")
                    nc.sync.dma_start(
                        out=r_sb[:, :T_slot], in_=r_d.ap()[3 * s : 3 * s + 3, :T_slot]
                    )
                    for rtile, rows in enumerate(tiles):
                        a_sb = apool.tile([PART, T_MAX], fp32, tag="a")
                        nc.sync.dma_start(
                            out=a_sb[:rows, :T_slot],
                            in_=a_d.ap()[
                                row0 + rtile * PART : row0 + rtile * PART + rows,
                                :T_slot,
                            ],
                        )
                        for c0 in range(0, T_slot, CHUNK):
                            w = min(CHUNK, T_slot - c0)
                            sq = sqp.tile([PART, CHUNK], fp32, tag="sq")
                            nc.tensor.matmul(
                                sq[:rows, :w],
                                l_sb[:, rtile * PART : rtile * PART + rows],
                                r_sb[:, c0 : c0 + w],
                                start=True,
                                stop=True,
                                skip_group_check=True,
                            )
                            e = epool.tile([PART, CHUNK], fp32, tag="e")
                            nc.scalar.activation(
                                e[:rows, :w],
                                sq[:rows, :w],
                                mybir.ActivationFunctionType.Exp,
                                bias=0.0,
                                scale=-1.0,
                            )
                            m = mpool.tile([PART, CHUNK], fp32, tag="m")
                            # m = (e - 1) * a ; acc column = row sums of m
                            nc.vector.scalar_tensor_tensor(
                                out=m[:rows, :w],
                                in0=e[:rows, :w],
                                scalar=1.0,
                                in1=a_sb[:rows, c0 : c0 + w],
                                op0=mybir.AluOpType.subtract,
                                op1=mybir.AluOpType.mult,
                                accum_out=acc[:rows, unit : unit + 1],
                            )
                            unit += 1
                    row0 += sum(tiles)
            nc.sync.dma_start(out=oacc_d.ap()[:], in_=acc[:])

    nc.compile()
    return nc


def _reduce_outputs(results):
    tot = 0.0
    for res in results:
        tot += np.asarray(res["out_acc"], dtype=np.float64).sum()
    loss = -tot / float(B * N_MAX * T_MAX)
    return np.array(loss, dtype=np.float32)


def kernel(alignments, input_lengths, target_lengths):
    from concourse.bass_utils import run_bass_kernel_spmd

    slots = _plan(input_lengths, target_lengths)
    in_maps = _host_inputs(alignments, input_lengths, target_lengths, slots)
    nc = _build_bass(slots, reps=1)
    out = run_bass_kernel_spmd(nc, in_maps, core_ids=list(range(N_CORES)))
    return _reduce_outputs(out.results)


if __name__ == "__main__":
    rng = np.random.default_rng(0)
    al = rng.random((B, N_MAX, T_MAX), dtype=np.float32)
    il = rng.integers(N_MAX // 2, N_MAX + 1, size=B).astype(np.int32)
    tl = rng.integers(T_MAX // 2, T_MAX + 1, size=B).astype(np.int32)
    print(kernel(alignments=al, input_lengths=il, target_lengths=tl))

